# revision 1
# baseline (speedup 1.0000x reference)
"""Trainium2 Bass kernel for nn_MultiHeadAttention (B=4, S=2048, DIM=768,
EMBED=512, HEADS=8, HEAD_DIM=64), distributed over 8 NeuronCores.

Sharding: core (b, g) with b in 0..3 (batch, data parallel) and g in 0..1
(head-group of 4 heads, tensor parallel). Each core computes a partial
output Y_partial[b,g] = softmax(QK^T/8) V @ Wo[g-slice]; the host sums the
two group partials per batch and adds the output bias.

Device dataflow per core (bf16 matmuls, fp32 PSUM accumulation):
  - host supplies x^T (DIM on partitions) so no on-device transposes exist
  - Q^T, K^T = Wg^T @ x^T        -> [256, S] layout, head_dim on partitions
  - V        = x @ Wv_g          -> [S, 256] natural layout
  - S^T      = (QK^T)^T via lhsT=K^T tile, rhs=Q^T tile; the two heads of
               an e-chunk run as concurrent K=64 PE tiles (row packing)
  - exp      = ScalarE ACTIVATE(Exp, scale=1/8) straight out of PSUM,
               FD=1024 per instruction; ScalarE is the saturated engine
               (~128us of exp) and sets the attention cadence
  - U^T;R    = PV matmul with lhsT=[V_h | ones64] (or [ones64 | V_h]) so
               the same matmul emits the softmax denominator, replicated
               64x, partition-aligned with the other head's slot
  - O^T      = U^T * (1/R): reciprocal by 2-step Newton from a constant
               seed on DVE (all plain tensor ops), ~1e-6 relative
  - Y        = O^T.T @ Wo -> natural [S, DIM], DMA out
Scheduling: projections, out-projection (in half-s-chunk units) and the
next block's Q^T projection are interleaved into the exp-bound attention
loops so the PE's spare issue slots absorb them.  A post-pass splits
multi-semaphore waits and the gpsimd RANGE_CLEAR into single-wait NoOps
for this image's stricter walrus.
"""

import numpy as np
import ml_dtypes

import concourse.bass as bass
import concourse.tile as tile
from concourse import mybir
from concourse.bass_utils import run_bass_kernel_spmd

BF16 = mybir.dt.bfloat16
F32 = mybir.dt.float32
FP8 = mybir.dt.float8e4
NPBF16 = ml_dtypes.bfloat16

B, S, DIM, EMBED, HEADS, HEAD_DIM = 4, 2048, 768, 512, 8, 64
P = 128
KD = DIM // P          # 6   contraction chunks for projections
GROUPS = 2             # head-groups (tensor-parallel split)
GE = EMBED // GROUPS   # 256 embed columns per group
GH = HEADS // GROUPS   # 4   heads per group
MQ = GE // P           # 2   e-chunks per group
SC = S // P            # 16  sequence chunks of 128
NB = 512               # matmul free-dim block
NQ = S // NB           # 4   query blocks
SCALE = 0.125          # 1/sqrt(HEAD_DIM)
NCORES = B * GROUPS    # 8
X0 = 1.0 / 2146.0      # Newton seed for 1/rowsum


def _split_multi_waits(nc):
    """The walrus build in this image accepts at most ONE sem-wait per
    instruction (setupSyncWait: 'Too many sync wait commands'), while Tile
    freely attaches several.  Hoist all but the last wait of each
    instruction onto same-engine NoOps inserted immediately before it —
    identical blocking semantics, one wait per instruction."""
    ctr = 0
    for f in nc.m.functions:
        for blk in f.blocks:
            il = blk.instructions
            out = []
            for inst in il:
                if type(inst).__name__ == "InstISA":
                    # kernel-tail gpsimd.sem_clear (RANGE_CLEAR): this
                    # walrus rejects its encoding ("ISA wrong length").
                    # NRT re-initializes semaphore state per execution, so
                    # replace it with a NoOp carrying the same syncs.
                    nop = mybir.InstNoOp(
                        name=f"{inst.name}-isanop", ins=[], outs=[]
                    )
                    nop.engine = inst.engine
                    nop.sync_info = inst.sync_info
                    out.append(nop)
                    continue
                si = inst.sync_info
                if si is not None and si.on_wait and len(si.on_wait) > 1:
                    waits = list(si.on_wait)
                    for w in waits[:-1]:
                        ctr += 1
                        nop = mybir.InstNoOp(
                            name=f"I-waitsplit-{ctr}", ins=[], outs=[]
                        )
                        nop.engine = inst.engine
                        nop.sync_info = mybir.SyncInfo(on_wait=[w], on_update=[])
                        out.append(nop)
                    si.on_wait = [waits[-1]]
                out.append(inst)
            il[:] = out
    return ctr


def build_nc(split_waits=True):
    nc = bass.Bass("TRN2", target_bir_lowering=False, debug=False)

    xqT = nc.dram_tensor("xqT", [DIM, S], BF16, kind="ExternalInput").ap()
    xkT = nc.dram_tensor("xkT", [DIM, S], BF16, kind="ExternalInput").ap()
    xvT = nc.dram_tensor("xvT", [DIM, S], BF16, kind="ExternalInput").ap()
    wq = nc.dram_tensor("wq", [DIM, GE], BF16, kind="ExternalInput").ap()
    wk = nc.dram_tensor("wk", [DIM, GE], BF16, kind="ExternalInput").ap()
    wv = nc.dram_tensor("wv", [DIM, GE], BF16, kind="ExternalInput").ap()
    wo = nc.dram_tensor("wo", [GE, DIM], BF16, kind="ExternalInput").ap()
    bq = nc.dram_tensor("bq", [GE], F32, kind="ExternalInput").ap()
    bk = nc.dram_tensor("bk", [GE], F32, kind="ExternalInput").ap()
    bv = nc.dram_tensor("bv", [GE], F32, kind="ExternalInput").ap()
    out = nc.dram_tensor("out", [S, DIM], F32, kind="ExternalOutput").ap()

    add = mybir.AluOpType.add
    mult = mybir.AluOpType.mult
    Exp = mybir.ActivationFunctionType.Exp


    with tile.TileContext(nc) as tc:
        with (
            tc.tile_pool(name="const", bufs=1) as const,
            # PSUM: "s" = 2 slots x [P,2,NB] (score pairs, 4 banks);
            #       "u" = 4 slots x 1 bank (proj blocks, PV accumulators,
            #             out-proj halves) = 8 banks total.
            tc.tile_pool(name="psS", bufs=2, space="PSUM") as psS,
            tc.tile_pool(name="psU", bufs=4, space="PSUM") as psU,
            tc.tile_pool(name="esp", bufs=4) as esp,
            tc.tile_pool(name="nrm", bufs=3) as nrm,
            tc.tile_pool(name="yout", bufs=2) as yout,
            tc.tile_pool(name="xin", bufs=3) as xin,
        ):
            wq_sb = const.tile([P, KD, GE], BF16, tag="wq")
            wk_sb = const.tile([P, KD, GE], BF16, tag="wk")
            wv_sb = const.tile([P, KD, GE], BF16, tag="wv")
            wo_sb = const.tile([P, MQ, DIM], BF16, tag="wo")
            bq_sb = const.tile([P, MQ], F32, tag="bq")
            bk_sb = const.tile([P, MQ], F32, tag="bk")
            bvb_sb = const.tile([P, GE], F32, tag="bvb")
            qt_sb = const.tile([P, MQ, S], BF16, tag="qt")   # Q^T
            kt_sb = const.tile([P, MQ, S], BF16, tag="kt")   # K^T
            ot_sb = const.tile([P, MQ, S], BF16, tag="ot")   # O^T
            # V in PV-lhsT layout: per (s-chunk, head) a [128, 128] block
            # of [V_h | ones] (even local head) or [ones | V_h] (odd); the
            # ones columns make the PV matmul also produce the softmax
            # denominator (replicated 64x) in the other partition half.
            v_sb = const.tile([P, SC, GH, P], BF16, tag="v")
            nc.vector.memset(v_sb[:], 1.0)

            # K first (attention needs all of K^T before its first matmul),
            # then Q block 0, then V; weights on the sync ring, activations
            # on the gpsimd ring so the loads overlap.
            nc.sync.dma_start(wk_sb[:], wk.rearrange("(k p) e -> p k e", p=P))
            nc.sync.dma_start(bk_sb[:], bk.rearrange("(m p) -> p m", p=P))
            nc.sync.dma_start(wq_sb[:], wq.rearrange("(k p) e -> p k e", p=P))
            nc.sync.dma_start(bq_sb[:], bq.rearrange("(m p) -> p m", p=P))
            nc.sync.dma_start(wv_sb[:], wv.rearrange("(k p) e -> p k e", p=P))
            nc.sync.dma_start(bvb_sb[:], bv.partition_broadcast(P))
            nc.sync.dma_start(wo_sb[:], wo.rearrange("(m p) d -> p m d", p=P))
            xk_sb = xin.tile([P, KD, S], BF16, tag="x", name="xk")
            xq_sb = xin.tile([P, KD, S], BF16, tag="x", name="xq")
            xv_sb = xin.tile([P, KD, S], BF16, tag="x", name="xv")
            for k in range(KD):
                nc.gpsimd.dma_start(xk_sb[:, k, :], xkT[k * P:(k + 1) * P, :])
            for k in range(KD):
                nc.gpsimd.dma_start(xq_sb[:, k, :], xqT[k * P:(k + 1) * P, :])
            for k in range(KD):
                nc.gpsimd.dma_start(xv_sb[:, k, :], xvT[k * P:(k + 1) * P, :])

            def qk_proj_block(x_sb, w_sb, b_sb, dst, m, n):
                ps = psU.tile([P, NB], F32, tag="u", name=f"pj{dst.name}_{m}_{n}")
                for k in range(KD):
                    nc.tensor.matmul(
                        ps[:],
                        lhsT=w_sb[:, k, m * P:(m + 1) * P],
                        rhs=x_sb[:, k, n * NB:(n + 1) * NB],
                        start=(k == 0),
                        stop=(k == KD - 1),
                    )
                nc.vector.tensor_scalar(
                    out=dst[:, m, n * NB:(n + 1) * NB],
                    in0=ps[:],
                    scalar1=b_sb[:, m:m + 1],
                    scalar2=None,
                    op0=add,
                )

            # ---- K^T fully, Q^T block 0 ----
            for n in range(NQ):
                for m in range(MQ):
                    qk_proj_block(xk_sb, wk_sb, bk_sb, kt_sb, m, n)
            for m in range(MQ):
                qk_proj_block(xq_sb, wq_sb, bq_sb, qt_sb, m, 0)
            # claim the first attention accumulators BEFORE the V loop so
            # attention q0 can start while V is still projecting
            pu_first = [
                psU.tile([P, NB], F32, tag="u", name=f"puF_{j}")
                for j in range(2)
            ]
            def v_proj_chunk(s):
                ps = psU.tile([P, GE], F32, tag="u", name=f"pv{s}")
                for k in range(KD):
                    nc.tensor.matmul(
                        ps[:],
                        lhsT=xv_sb[:, k, s * P:(s + 1) * P],
                        rhs=wv_sb[:, k, :],
                        start=(k == 0),
                        stop=(k == KD - 1),
                    )
                ps_h = ps.rearrange("p (h d) -> p h d", d=HEAD_DIM)
                bv_h = bvb_sb.rearrange("p (h d) -> p h d", d=HEAD_DIM)
                # even local heads -> cols [0:64], odd -> cols [64:128]
                nc.vector.tensor_tensor(
                    out=v_sb[:, s, 0::2, 0:HEAD_DIM],
                    in0=ps_h[:, 0::2, :], in1=bv_h[:, 0::2, :], op=add,
                )
                nc.vector.tensor_tensor(
                    out=v_sb[:, s, 1::2, HEAD_DIM:P],
                    in0=ps_h[:, 1::2, :], in1=bv_h[:, 1::2, :], op=add,
                )
            for s in range(SC):
                v_proj_chunk(s)

            # out-projection in half-s-chunk units (2 matmuls + 1 copy),
            # spread through later attention loops' PE slack.
            def out_proj_unit(s, half):
                lo, hi = (0, NB) if half == 0 else (NB, DIM)
                py = psU.tile([P, NB], F32, tag="u", name=f"py{s}_{half}")
                for k in range(MQ):
                    nc.tensor.matmul(
                        py[:, 0:hi - lo],
                        lhsT=ot_sb[:, k, s * P:(s + 1) * P],
                        rhs=wo_sb[:, k, lo:hi],
                        start=(k == 0),
                        stop=(k == MQ - 1),
                    )
                if half == 0:
                    out_proj_unit.y[s] = yout.tile([P, DIM], F32, tag="y",
                                                   name=f"y{s}")
                y_sb = out_proj_unit.y[s]
                nc.vector.tensor_copy(y_sb[:, lo:hi], py[:, 0:hi - lo])
                if half == 1:
                    nc.sync.dma_start(out[s * P:(s + 1) * P, :], y_sb[:])
            out_proj_unit.y = {}
            out_proj_unit.todo = 0

            def drain_out_proj(limit):
                if out_proj_unit.todo < limit:
                    unit = out_proj_unit.todo
                    out_proj_unit(unit // 2, unit % 2)
                    out_proj_unit.todo = unit + 1

            def make_normalize(pu, hp, q):
                def _norm():
                    for j in range(2):
                        # U^T on rows [j*64, +64); replicated rowsum on the
                        # other half.  One copy frees the PV bank; 1/rowsum
                        # via 2-step Newton from a constant seed (~1e-6 rel).
                        ulo, uhi = j * HEAD_DIM, (j + 1) * HEAD_DIM
                        rlo, rhi = (1 - j) * HEAD_DIM, (2 - j) * HEAD_DIM
                        ur = nrm.tile([P, NB], F32, tag=f"ur{j}",
                                      name=f"ur{hp}_{q}_{j}")
                        nc.vector.tensor_copy(ur[:], pu[j][:])
                        rr = ur[rlo:rhi, :]
                        x1 = nrm.tile([P, NB], F32, tag="x1")
                        tmp = nrm.tile([P, NB], F32, tag="tmp")
                        nc.vector.tensor_scalar(       # x1 = 2x0 - x0^2 r
                            out=x1[rlo:rhi, :], in0=rr,
                            scalar1=-X0 * X0, scalar2=2.0 * X0,
                            op0=mult, op1=add,
                        )
                        nc.vector.tensor_tensor(       # e = r * x1
                            out=tmp[rlo:rhi, :], in0=rr,
                            in1=x1[rlo:rhi, :], op=mult,
                        )
                        nc.vector.tensor_scalar(       # u = 2 - e
                            out=tmp[rlo:rhi, :], in0=tmp[rlo:rhi, :],
                            scalar1=-1.0, scalar2=2.0,
                            op0=mult, op1=add,
                        )
                        nc.vector.tensor_tensor(       # x2 = x1 * u
                            out=x1[rlo:rhi, :], in0=x1[rlo:rhi, :],
                            in1=tmp[rlo:rhi, :], op=mult,
                        )
                        # recip rows onto U partitions, then scale into O^T
                        nc.sync.dma_start(x1[ulo:uhi, :], x1[rlo:rhi, :])
                        nc.vector.tensor_tensor(
                            out=ot_sb[ulo:uhi, hp, q * NB:(q + 1) * NB],
                            in0=ur[ulo:uhi, :],
                            in1=x1[ulo:uhi, :],
                            op=mult,
                        )
                return _norm
            pend = []
            qk_partial = {}

            # ---- attention, one q block at a time ----
            for q in range(NQ):
                for hp in range(MQ):          # head pair == e-chunk
                    if q == 0 and hp == 0:
                        pu = pu_first
                    else:
                        pu = [
                            psU.tile([P, NB], F32, tag="u",
                                     name=f"pu{hp}_{q}_{j}")
                            for j in range(2)
                        ]
                    for m in range(SC):       # key chunk of 128
                        ss = psS.tile([P, 2, NB], F32, tag="s")
                        for j in range(2):
                            lo, hi = j * HEAD_DIM, (j + 1) * HEAD_DIM
                            nc.tensor.matmul(
                                ss[:, j, :],
                                lhsT=kt_sb[lo:hi, hp, m * P:(m + 1) * P],
                                rhs=qt_sb[lo:hi, hp, q * NB:(q + 1) * NB],
                                start=True,
                                stop=True,
                            )
                        es = esp.tile([P, 2, NB], BF16, tag="es")
                        nc.scalar.activation(es[:], ss[:], Exp, scale=SCALE)
                        for j in range(2):
                            nc.tensor.matmul(
                                pu[j][:],
                                lhsT=v_sb[:, m, 2 * hp + j, :],
                                rhs=es[:, j, :],
                                start=(m == 0),
                                stop=(m == SC - 1),
                            )
                        # previous block's deferred normalize: emit at the
                        # start of this loop — it is DVE/sync-only work, and
                        # its leading copies release the PV banks this
                        # block's first PV matmuls are waiting on
                        if m == 0 and pend:
                            pend.pop(0)()
                        # one out-proj half-unit every other chunk, once the
                        # previous q block's O^T rows exist
                        if m % 2 == 1 and m >= 5:
                            drain_out_proj(8 * q)
                        # next q block's Q^T projection: 2 matmuls at a
                        # time on even iterations so no single iteration
                        # carries a full 1.3us block
                        if hp == 1 and q + 1 < NQ and m in (4, 6, 8, 10, 12, 14):
                            mq_idx, phase = (0, (m - 4) // 2) if m <= 8 else (1, (m - 10) // 2)
                            if phase == 0:
                                qp = psU.tile([P, NB], F32, tag="u",
                                              name=f"qp{q + 1}_{mq_idx}")
                                qk_partial[mq_idx] = qp
                            qp = qk_partial[mq_idx]
                            for k in (2 * phase, 2 * phase + 1):
                                nc.tensor.matmul(
                                    qp[:],
                                    lhsT=wq_sb[:, k, mq_idx * P:(mq_idx + 1) * P],
                                    rhs=xq_sb[:, k, (q + 1) * NB:(q + 2) * NB],
                                    start=(k == 0),
                                    stop=(k == KD - 1),
                                )
                            if phase == 2:
                                nc.vector.tensor_scalar(
                                    out=qt_sb[:, mq_idx, (q + 1) * NB:(q + 2) * NB],
                                    in0=qp[:],
                                    scalar1=bq_sb[:, mq_idx:mq_idx + 1],
                                    scalar2=None,
                                    op0=add,
                                )
                    pend.append(make_normalize(pu, hp, q))
            for th in pend:
                th()
            pend = []
            while out_proj_unit.todo < 8 * NQ:
                unit = out_proj_unit.todo
                out_proj_unit(unit // 2, unit % 2)
                out_proj_unit.todo = unit + 1

    if split_waits:
        _split_multi_waits(nc)
    return nc


_NC = None


def _get_nc():
    global _NC
    if _NC is None:
        _NC = build_nc()
    return _NC


def _bf(a):
    return np.ascontiguousarray(np.asarray(a, dtype=np.float32)).astype(NPBF16)


def make_in_maps(query, key, value, wq, bq, wk, bk, wv, bv, wo, bo):
    query = np.asarray(query, np.float32)
    key = np.asarray(key, np.float32)
    value = np.asarray(value, np.float32)
    wq = np.asarray(wq, np.float32)
    wk = np.asarray(wk, np.float32)
    wv = np.asarray(wv, np.float32)
    wo = np.asarray(wo, np.float32)
    in_maps = []
    for b in range(B):
        xqT = _bf(query[b].T)
        xkT = _bf(key[b].T)
        xvT = _bf(value[b].T)
        for g in range(GROUPS):
            sl = slice(g * GE, (g + 1) * GE)
            in_maps.append({
                "xqT": xqT,
                "xkT": xkT,
                "xvT": xvT,
                "wq": _bf(wq[:, sl]),
                "wk": _bf(wk[:, sl]),
                "wv": _bf(wv[:, sl]),
                "wo": _bf(wo[sl, :]),
                "bq": np.ascontiguousarray(np.asarray(bq, np.float32)[sl]),
                "bk": np.ascontiguousarray(np.asarray(bk, np.float32)[sl]),
                "bv": np.ascontiguousarray(np.asarray(bv, np.float32)[sl]),
            })
    return in_maps


def kernel(query, key, value, wq, bq, wk, bk, wv, bv, wo, bo, **kw):
    nc = _get_nc()
    in_maps = make_in_maps(query, key, value, wq, bq, wk, bk, wv, bv, wo, bo)
    res = run_bass_kernel_spmd(nc, in_maps, list(range(NCORES))).results
    bo = np.asarray(bo, np.float32)
    out = np.empty((B, S, DIM), np.float32)
    for b in range(B):
        out[b] = res[b * GROUPS]["out"] + res[b * GROUPS + 1]["out"] + bo
    return out



# revision 5
# speedup vs baseline: 1.1673x; 1.1673x over previous
"""Trainium2 Bass kernel for nn_MultiHeadAttention (B=4, S=2048, DIM=768,
EMBED=512, HEADS=8, HEAD_DIM=64), distributed over 8 NeuronCores.

Sharding: core (b, g), b in 0..3 (batch), g in 0..1 (head-group of 4 heads).
Host sums the two group partials per batch and adds the effective output
bias (bo + bv @ wo; bk is dropped — a per-row-constant logit shift is
softmax-invariant).

v3 (bf16 everywhere; fp8 was 4x over the error budget):
  - PV chunks are emitted through a global lagged work queue so the
    in-order PE queue never head-blocks on ACT (v1 lost ~1.1us/iter).
  - The PV accumulator (single PSUM buffer) is released by 4 plain DVE
    copies (U and R halves) right after a block's last PV; the Newton
    reciprocal (gpsimd), partition-shift DMA (sync) and final multiply
    (DVE) run off the critical path.
  - exp on ACT, FD=1024 per key-chunk, es in an 8-slot bf16 ring.
  - Preamble: DMA rings ordered for earliest first exp (wk,wq then xk,
    xq query-block-0 slice, then xv, then the rest of xq); K proj
    m-major; V projections and block-0 PV drain in once xv lands.
  - Tail: last block's out-projection immediately after its normalize.
"""

import numpy as np
import ml_dtypes

import concourse.bass as bass
import concourse.tile as tile
from concourse import mybir
from concourse.bass_utils import run_bass_kernel_spmd

BF16 = mybir.dt.bfloat16
F32 = mybir.dt.float32
NPBF16 = ml_dtypes.bfloat16

B, S, DIM, EMBED, HEADS, HEAD_DIM = 4, 2048, 768, 512, 8, 64
P = 128
KD = DIM // P
GROUPS = 2
GE = EMBED // GROUPS   # 256
GH = HEADS // GROUPS   # 4
MQ = GE // P           # 2
SC = S // P            # 16
NB = 512
NQ = S // NB           # 4
SCALE = 0.125
NCORES = B * GROUPS
ESR = 8                # es ring slots
X0 = 1.0 / 2146.0      # Newton seed for 1/rowsum


def _split_multi_waits(nc):
    """This image's walrus accepts at most ONE sem-wait per instruction.
    Hoist all but the last wait onto same-engine NoOps; replace the tail
    gpsimd RANGE_CLEAR (rejected encoding) with a NoOp."""
    ctr = 0
    for f in nc.m.functions:
        for blk in f.blocks:
            il = blk.instructions
            out = []
            for inst in il:
                if type(inst).__name__ == "InstISA":
                    nop = mybir.InstNoOp(
                        name=f"{inst.name}-isanop", ins=[], outs=[]
                    )
                    nop.engine = inst.engine
                    nop.sync_info = inst.sync_info
                    out.append(nop)
                    continue
                si = inst.sync_info
                if si is not None and si.on_wait and len(si.on_wait) > 1:
                    waits = list(si.on_wait)
                    for w in waits[:-1]:
                        ctr += 1
                        nop = mybir.InstNoOp(
                            name=f"I-waitsplit-{ctr}", ins=[], outs=[]
                        )
                        nop.engine = inst.engine
                        nop.sync_info = mybir.SyncInfo(on_wait=[w], on_update=[])
                        out.append(nop)
                    si.on_wait = [waits[-1]]
                out.append(inst)
            il[:] = out
    return ctr


def build_nc(split_waits=True):
    nc = bass.Bass("TRN2", target_bir_lowering=False, debug=False)

    xqT = nc.dram_tensor("xqT", [DIM, S], BF16, kind="ExternalInput").ap()
    xkT = nc.dram_tensor("xkT", [DIM, S], BF16, kind="ExternalInput").ap()
    xvT = nc.dram_tensor("xvT", [DIM, S], BF16, kind="ExternalInput").ap()
    wq = nc.dram_tensor("wq", [DIM, GE], BF16, kind="ExternalInput").ap()
    wk = nc.dram_tensor("wk", [DIM, GE], BF16, kind="ExternalInput").ap()
    wv = nc.dram_tensor("wv", [DIM, GE], BF16, kind="ExternalInput").ap()
    wo = nc.dram_tensor("wo", [GE, DIM], BF16, kind="ExternalInput").ap()
    bq = nc.dram_tensor("bq", [GE], F32, kind="ExternalInput").ap()
    out = nc.dram_tensor("out", [S, DIM], F32, kind="ExternalOutput").ap()

    add = mybir.AluOpType.add
    mult = mybir.AluOpType.mult
    Exp = mybir.ActivationFunctionType.Exp

    with tile.TileContext(nc) as tc:
        with (
            tc.tile_pool(name="const", bufs=1) as const,
            tc.tile_pool(name="psS", bufs=2, space="PSUM") as psS,
            tc.tile_pool(name="psPU", bufs=1, space="PSUM") as psPU,
            tc.tile_pool(name="psM", bufs=2, space="PSUM") as psM,
            tc.tile_pool(name="nrm", bufs=2) as nrm,
            tc.tile_pool(name="yout", bufs=2) as yout,
        ):
            wq_sb = const.tile([P, KD, GE], BF16, tag="wq")
            wk_sb = const.tile([P, KD, GE], BF16, tag="wk")
            wv_sb = const.tile([P, KD, GE], BF16, tag="wv")
            wo_sb = const.tile([P, MQ, DIM], BF16, tag="wo")
            bq_sb = const.tile([P, MQ], F32, tag="bq")
            xq_sb = const.tile([P, KD, S], BF16, tag="xq")
            xk_sb = const.tile([P, KD, S], BF16, tag="xk")
            xv_sb = const.tile([P, KD, S], BF16, tag="xv")
            qt_sb = const.tile([P, MQ, S], BF16, tag="qt")
            kt_sb = const.tile([P, MQ, S], BF16, tag="kt")
            ot_sb = const.tile([P, MQ, S], BF16, tag="ot")
            es_sb = const.tile([P, ESR, 2, NB], BF16, tag="es")
            v_sb = const.tile([P, SC, GH, P], BF16, tag="v")
            # only the ones-halves need the memset; V halves get overwritten
            nc.vector.memset(v_sb[:, :, 0::2, HEAD_DIM:P], 1.0)
            nc.vector.memset(v_sb[:, :, 1::2, 0:HEAD_DIM], 1.0)

            # --- input DMAs.  sync: weights; gpsimd: xk, xq(q0 cols), xv,
            # xq(rest) — queue order is service order, so the first-exp
            # critical bytes (wk,wq,xk,xq[:,:512]) lead the bus.
            nc.sync.dma_start(wk_sb[:], wk.rearrange("(k p) e -> p k e", p=P))
            nc.sync.dma_start(wq_sb[:], wq.rearrange("(k p) e -> p k e", p=P))
            nc.sync.dma_start(bq_sb[:], bq.rearrange("(m p) -> p m", p=P))
            nc.sync.dma_start(wv_sb[:], wv.rearrange("(k p) e -> p k e", p=P))
            nc.sync.dma_start(wo_sb[:], wo.rearrange("(m p) d -> p m d", p=P))
            xkr = xkT.rearrange("(k p) s -> p k s", p=P)
            xqr = xqT.rearrange("(k p) s -> p k s", p=P)
            xvr = xvT.rearrange("(k p) s -> p k s", p=P)
            nc.gpsimd.dma_start(xk_sb[:], xkr)
            nc.gpsimd.dma_start(xq_sb[:, :, 0:NB], xqr[:, :, 0:NB])
            nc.gpsimd.dma_start(xv_sb[:], xvr)
            nc.gpsimd.dma_start(xq_sb[:, :, NB:S], xqr[:, :, NB:S])

            # --- building blocks ---
            def qk_proj_block(x_sb, w_sb, dst, m, n, with_bias):
                ps = psM.tile([P, NB], F32, tag="m", name=f"pj{dst.name}{m}_{n}")
                for k in range(KD):
                    nc.tensor.matmul(
                        ps[:],
                        lhsT=w_sb[:, k, m * P:(m + 1) * P],
                        rhs=x_sb[:, k, n * NB:(n + 1) * NB],
                        start=(k == 0), stop=(k == KD - 1),
                    )
                if with_bias:
                    nc.vector.tensor_scalar(
                        out=dst[:, m, n * NB:(n + 1) * NB], in0=ps[:],
                        scalar1=bq_sb[:, m:m + 1], scalar2=None, op0=add,
                    )
                else:
                    nc.vector.tensor_copy(dst[:, m, n * NB:(n + 1) * NB], ps[:])

            def v_proj_chunk(s):
                ps = psM.tile([P, NB], F32, tag="m", name=f"pv{s}")
                for k in range(KD):
                    nc.tensor.matmul(
                        ps[:, 0:GE],
                        lhsT=xv_sb[:, k, s * P:(s + 1) * P],
                        rhs=wv_sb[:, k, :],
                        start=(k == 0), stop=(k == KD - 1),
                    )
                ps_h = ps[:, 0:GE].rearrange("p (h d) -> p h d", d=HEAD_DIM)
                nc.vector.tensor_copy(v_sb[:, s, 0::2, 0:HEAD_DIM],
                                      ps_h[:, 0::2, :])
                nc.vector.tensor_copy(v_sb[:, s, 1::2, HEAD_DIM:P],
                                      ps_h[:, 1::2, :])

            def out_proj_half(s, half):
                lo, hi = (0, NB) if half == 0 else (NB, DIM)
                py = psM.tile([P, NB], F32, tag="m", name=f"py{s}_{half}")
                for k in range(MQ):
                    nc.tensor.matmul(
                        py[:, 0:hi - lo],
                        lhsT=ot_sb[:, k, s * P:(s + 1) * P],
                        rhs=wo_sb[:, k, lo:hi],
                        start=(k == 0), stop=(k == MQ - 1),
                    )
                if half == 0:
                    out_proj_half.y[s] = yout.tile([P, DIM], F32, tag="y",
                                                   name=f"y{s}")
                y_sb = out_proj_half.y[s]
                nc.vector.tensor_copy(y_sb[:, lo:hi], py[:, 0:hi - lo])
                if half == 1:
                    nc.sync.dma_start(out[s * P:(s + 1) * P, :], y_sb[:])
            out_proj_half.y = {}
            out_proj_half.done = {q: 0 for q in range(NQ)}

            def normalize(pu, hp, q):
                """Free pu via 4 DVE copies, then recip + multiply off the
                critical path (gpsimd Newton, sync shift-DMA, one DVE op)."""
                rt = nrm.tile([P, NB], F32, tag="rt", name=f"rt{hp}_{q}")
                uc = nrm.tile([P, NB], F32, tag="uc", name=f"uc{hp}_{q}")
                x1 = nrm.tile([P, NB], F32, tag="x1", name=f"x1{hp}_{q}")
                tm = nrm.tile([P, NB], F32, tag="tm", name=f"tm{hp}_{q}")
                xr = nrm.tile([P, NB], F32, tag="xr", name=f"xr{hp}_{q}")
                nc.vector.tensor_copy(uc[0:64, :], pu[0:64, 0, :])
                nc.vector.tensor_copy(uc[64:128, :], pu[64:128, 1, :])
                nc.vector.tensor_copy(rt[64:128, :], pu[64:128, 0, :])
                nc.vector.tensor_copy(rt[0:64, :], pu[0:64, 1, :])
                nc.gpsimd.tensor_scalar(   # x1 = 2x0 - x0^2 r
                    out=x1[:], in0=rt[:], scalar1=-X0 * X0,
                    scalar2=2.0 * X0, op0=mult, op1=add,
                )
                nc.gpsimd.tensor_tensor(   # e = r * x1
                    out=tm[:], in0=rt[:], in1=x1[:], op=mult,
                )
                nc.gpsimd.tensor_scalar(   # u = 2 - e
                    out=tm[:], in0=tm[:], scalar1=-1.0, scalar2=2.0,
                    op0=mult, op1=add,
                )
                nc.gpsimd.tensor_tensor(   # x2 = x1 * u
                    out=x1[:], in0=x1[:], in1=tm[:], op=mult,
                )
                nc.sync.dma_start(xr[0:64, :], x1[64:128, :])
                nc.sync.dma_start(xr[64:128, :], x1[0:64, :])
                qs = slice(q * NB, (q + 1) * NB)
                nc.vector.tensor_tensor(out=ot_sb[:, hp, qs], in0=uc[:],
                                        in1=xr[:], op=mult)

            # --- preamble projections ---
            for n in range(NQ):
                qk_proj_block(xk_sb, wk_sb, kt_sb, 0, n, False)
            qk_proj_block(xq_sb, wq_sb, qt_sb, 0, 0, True)
            qk_proj_block(xq_sb, wq_sb, qt_sb, 1, 0, True)

            # --- attention with global lagged PV queue ---
            state = {"pv": 0, "v": 0, "pu": {}, "oq": []}
            PVLAG, VLAG = 3, 6

            def emit_pv_chunk(i):
                pblk, c = divmod(i, SC)
                pq, php = divmod(pblk, MQ)
                if c == 0:
                    state["pu"][pblk] = psPU.tile([P, 2, NB], F32, tag="pu",
                                                  name=f"pu{pblk}")
                pu = state["pu"][pblk]
                for j in range(2):
                    nc.tensor.matmul(
                        pu[:, j, :],
                        lhsT=v_sb[:, c, 2 * php + j, :],
                        rhs=es_sb[:, c % ESR, j, :],
                        start=(c == 0), stop=(c == SC - 1),
                    )
                if c == SC - 1:
                    normalize(pu, php, pq)
                    del state["pu"][pblk]
                    if php == 1:
                        state["oq"].append(pq)

            def drain(gtime, max_pv):
                # V projections: one per tick once xv has landed
                if state["v"] < SC and gtime >= VLAG:
                    v_proj_chunk(state["v"])
                    state["v"] += 1
                n = 0
                while state["pv"] < NCORES * SC and n < max_pv:
                    i = state["pv"]
                    pblk, c = divmod(i, SC)
                    if SC * pblk + c > gtime - PVLAG:
                        break
                    if c >= state["v"]:
                        break
                    emit_pv_chunk(i)
                    state["pv"] += 1
                    n += 1

            for q in range(NQ):
                for hp in range(MQ):
                    blk = q * MQ + hp
                    for m in range(SC):
                        gtime = SC * blk + m
                        ss = psS.tile([P, 2, NB], F32, tag="s")
                        for j in range(2):
                            lo, hi = j * HEAD_DIM, (j + 1) * HEAD_DIM
                            nc.tensor.matmul(
                                ss[:, j, :],
                                lhsT=kt_sb[lo:hi, hp, m * P:(m + 1) * P],
                                rhs=qt_sb[lo:hi, hp, q * NB:(q + 1) * NB],
                                start=True, stop=True,
                            )
                        nc.scalar.activation(es_sb[:, m % ESR, :, :], ss[:],
                                             Exp, scale=SCALE)
                        if blk == 0 and 1 <= m <= 4:
                            qk_proj_block(xk_sb, wk_sb, kt_sb, 1, m - 1, False)
                        if hp == 1 and q + 1 < NQ and m in (6, 12):
                            qk_proj_block(xq_sb, wq_sb, qt_sb,
                                          m // 12, q + 1, True)
                        drain(gtime, 2)
                        if m % 2 == 1 and m >= 5 and state["oq"]:
                            qd = state["oq"][0]
                            if out_proj_half.done[qd] < 8:
                                nn = out_proj_half.done[qd]
                                out_proj_half(qd * 4 + nn // 2, nn % 2)
                                out_proj_half.done[qd] = nn + 1
                            else:
                                state["oq"].pop(0)

            # --- tail: remaining PV chunks, normalizes, out-projections ---
            gtime = NCORES * SC
            while state["pv"] < NCORES * SC:
                drain(gtime, 2)
                gtime += 1
            for q in range(NQ):
                while out_proj_half.done[q] < 8:
                    nn = out_proj_half.done[q]
                    out_proj_half(q * 4 + nn // 2, nn % 2)
                    out_proj_half.done[q] = nn + 1

    if split_waits:
        _split_multi_waits(nc)
    return nc


_NC = None


def _get_nc():
    global _NC
    if _NC is None:
        _NC = build_nc()
    return _NC


def _bf(a):
    return np.ascontiguousarray(np.asarray(a, dtype=np.float32)).astype(NPBF16)


def make_in_maps(query, key, value, wq, bq, wk, bk, wv, bv, wo, bo):
    query = np.asarray(query, np.float32)
    key = np.asarray(key, np.float32)
    value = np.asarray(value, np.float32)
    wqf = np.asarray(wq, np.float32)
    wkf = np.asarray(wk, np.float32)
    wvf = np.asarray(wv, np.float32)
    wof = np.asarray(wo, np.float32)
    in_maps = []
    for b in range(B):
        xqT = _bf(query[b].T)
        xkT = _bf(key[b].T)
        xvT = _bf(value[b].T)
        for g in range(GROUPS):
            sl = slice(g * GE, (g + 1) * GE)
            in_maps.append({
                "xqT": xqT,
                "xkT": xkT,
                "xvT": xvT,
                "wq": _bf(wqf[:, sl]),
                "wk": _bf(wkf[:, sl]),
                "wv": _bf(wvf[:, sl]),
                "wo": _bf(wof[sl, :]),
                "bq": np.ascontiguousarray(np.asarray(bq, np.float32)[sl]),
            })
    return in_maps


def kernel(query, key, value, wq, bq, wk, bk, wv, bv, wo, bo, **kw):
    nc = _get_nc()
    in_maps = make_in_maps(query, key, value, wq, bq, wk, bk, wv, bv, wo, bo)
    res = run_bass_kernel_spmd(nc, in_maps, list(range(NCORES))).results
    # bk is softmax-invariant; bv rides through softmax into a constant
    bo_eff = (np.asarray(bo, np.float32)
              + np.asarray(bv, np.float32) @ np.asarray(wo, np.float32))
    outp = np.empty((B, S, DIM), np.float32)
    for b in range(B):
        outp[b] = res[b * GROUPS]["out"] + res[b * GROUPS + 1]["out"] + bo_eff
    return outp
